# revision 5
# baseline (speedup 1.0000x reference)
"""Dense transformer (B=4,T=1024,C=1024,H=16,L=8) on 8 TRN2 NeuronCores.

Sharding: DP over batch (4) x sequence block-cyclic (2). Core c = 2b+s handles
batch b, token blocks {j : j%2==s} (128 tokens each, 512 tokens/core). Only
attention needs cross-core data: per layer, K/V are exchanged between pair
cores via two 2-rank AllGathers (K first, then V), overlapped with the Q
projection and the attention pass over local K/V.

Layouts: activations live transposed [C, T_local] (channels on partitions) so
the whole layer chain runs with zero transposes. LayerNorm gains/biases/means
are folded into the weights on the host (rank-1 correction); the per-token
rstd is applied POST-matmul at eviction time.

fp8: the six weight-stationary GEMMs (QKV, O-proj, FC1, FC2) run in fp8e4m3
with MatmulPerfMode.DoubleRow — each instruction contracts a PAIR of
128-channel k-tiles at double rate. Weights are scaled by a per-class power
of two on the host (sigma~0.02 would land in fp8 subnormals); descales are
folded into existing instructions: Exp's scale absorbs Q&K's weight scales,
the softmax-denominator reciprocal absorbs V's, the LN-rstd sqrt absorbs
FC1's, and scalar_tensor_tensor mult slots absorb O/FC2's. Attention
(QK/AV), the embed and the small head matmuls stay bf16; the residual is
carried f32r and all matmuls accumulate in fp32 PSUM.
"""
import sys
import os
import numpy as np
import ml_dtypes

sys.path.insert(0, '/opt/trn_rl_repo')

import concourse.bass as bass  # noqa: F401
import concourse.tile as tile
from concourse import bacc, mybir
from concourse.bass_utils import run_bass_kernel_spmd

F = mybir.ActivationFunctionType
ALU = mybir.AluOpType
PM = mybir.MatmulPerfMode
dt = mybir.dt
AF32 = np.float32
ABF16 = ml_dtypes.bfloat16
AE4 = ml_dtypes.float8_e4m3fn

B, T, C, H, L = 4, 1024, 1024, 16, 8
DIN, DOUT, DH = 128, 256, 64
TL = 512
NB = 4
NCH = C // 128
NPR = NCH // 2
DFF = 4 * C
EPS = 1e-5
KW = NCH * TL
VW = NB * (H * 65)

_CACHE = {}
L_RUN = int(os.environ.get('KLAYERS', str(L)))

# activation pre-scale exponents (pow2) applied before fp8 casts to keep
# small values out of e4m3 subnormal range; descale folds into existing
# eviction constants.
AH = 3   # residual h8
AX = 3   # normalized x8 (V-proj stationary)
AY = 5   # attention output y8

# layers whose MLP runs fp8 DoubleRow; the rest run bf16. fp8 MLP noise
# dominates the error budget, and EARLY layer noise is ~4x more damaging
# than late (it compounds through the stack) — so late layers go fp8.
MLP8 = frozenset(
    int(x) for x in os.environ.get('KMLP8', '3,4,5,6,7').split(',') if x != '')


def _build(use_bias, sq, sk, sv, so, s1, s2, mlp8):
    """use_bias: dict of bools; s*: power-of-2 exponents of the fp8 weight
    scaling per matrix class (weights stored as W*2^s); mlp8: layer indices
    whose MLP runs fp8 DoubleRow (others run bf16 for accuracy)."""
    nc = bacc.Bacc("TRN2", target_bir_lowering=False, debug=False, num_devices=8)
    l16 = sorted(set(range(L)) - set(mlp8))
    l16idx = {li: i for i, li in enumerate(l16)}

    def par(name, shape, dtp):
        return nc.declare_dram_parameter(name, list(shape), dtp, isOutput=False)

    xloc = par('xloc', [128, TL], dt.bfloat16)
    xshift = par('xshift', [128, TL], dt.float32)
    emb_w = par('emb_w', [128, C], dt.bfloat16)
    emb_b = par('emb_b', [128, NCH], dt.float32)
    wq = par('wq', [L, NPR, 128, 2, C], dt.float8e4)
    wk = par('wk', [L, NPR, 128, 2, C], dt.float8e4)
    wv = par('wv', [L, NPR, 128, 2, C], dt.float8e4)
    bq = par('bq', [L, 128, NCH], dt.float32)
    bk = par('bk', [L, 128, NCH], dt.float32)
    bv = par('bv', [L, 128, NCH], dt.float32)
    wo = par('wo', [L, NPR, 128, 2, C], dt.float8e4)
    bo = par('bo', [L, 128, NCH], dt.float32)
    w1 = par('w1', [L, NPR, 4, 128, 2, 1024], dt.float8e4)
    b1 = par('b1', [L, 128, 32], dt.float32)
    w2 = par('w2', [L, DFF // 256, 128, 2, C], dt.float8e4)
    b2 = par('b2', [L, 128, NCH], dt.float32)
    if l16:
        w1b = par('w1b', [len(l16), C, DFF], dt.bfloat16)
        w2b = par('w2b', [len(l16), DFF, C], dt.bfloat16)
    pw = par('pw', [C, DOUT], dt.bfloat16)
    pb = par('pb', [128, 2], dt.float32)
    dw1 = par('dw1', [DOUT, DOUT], dt.bfloat16)
    db1 = par('db1', [128, 2], dt.float32)
    dw2 = par('dw2', [DOUT, DIN], dt.bfloat16)
    db2 = par('db2', [128, 1], dt.float32)
    mask_tri = par('mask_tri', [128, 128], dt.bfloat16)
    mask_p0 = par('mask_p0', [128, 128], dt.bfloat16)
    selp = par('selp', [128, 2], dt.float32)
    out_p = nc.declare_dram_parameter('out', [128, TL], dt.float32, isOutput=True)

    exp_scale = float(2.0 ** (-(sq + sk)))

    with tile.TileContext(nc, num_cores=8) as tc:
        with tc.tile_pool(name='persist', bufs=1) as pp, \
             tc.tile_pool(name='sbwork', bufs=1) as wkp, \
             tc.tile_pool(name='wslab', bufs=1) as wsp, \
             tc.tile_pool(name='small', bufs=1) as smp, \
             tc.tile_pool(name='dram', bufs=2, space='DRAM') as drp:

            hT = pp.tile([128, NCH, TL], dt.float32r, name='hT')
            h8 = pp.tile([128, NCH, TL], dt.float8e4, name='h8')
            QT = pp.tile([128, NCH, TL], dt.bfloat16, name='QT')
            KTl = pp.tile([128, NCH, TL], dt.bfloat16, name='KTl')
            # V stored head-group-major [g, j, hh, 128]: cols 0:64 are V
            # data, cols 64:128 are ones so the AV matmul emits the softmax
            # denominator replicated on PSUM partitions 64:128 (no gpsimd
            # partition_broadcast needed). Only cols 0:64 are exchanged.
            Vl = pp.tile([128, 4, NB, 4, 128], dt.bfloat16, name='Vl')
            Vpm = pp.tile([128, 4, NB, 4, 128], dt.bfloat16, name='Vpm')
            peer = pp.tile([128, KW], dt.bfloat16, name='peer')
            KTp = peer[:, 0:KW].rearrange("p (c t) -> p c t", c=NCH)
            y8 = pp.tile([128, NCH, TL], dt.float8e4, name='y8')
            mtri = pp.tile([128, 128], dt.bfloat16, name='mtri')
            mp0 = pp.tile([128, 128], dt.bfloat16, name='mp0')
            selc = pp.tile([128, 2], dt.float32, name='selc')
            ones_mf = pp.tile([128, 128], dt.float32, name='ones_mf')
            ones_mat = pp.tile([128, 128], dt.float32r, name='ones_mat')
            ones_f32 = pp.tile([128, 1], dt.float32, name='ones_f32')
            eps_t = pp.tile([128, 1], dt.float32, name='eps_t')
            eps_s1 = pp.tile([128, 1], dt.float32, name='eps_s1')
            eps_ah = pp.tile([128, 1], dt.float32, name='eps_ah')
            dscO = pp.tile([128, 1], dt.float32, name='dscO')
            dscW2 = pp.tile([128, 1], dt.float32, name='dscW2')
            cAX = pp.tile([128, 1], dt.float32, name='cAX')
            xl_sb = pp.tile([128, TL], dt.bfloat16, name='xl_sb')
            xsh_sb = pp.tile([128, TL], dt.float32, name='xsh_sb')
            featsT = pp.tile([128, 2, TL], dt.bfloat16, name='featsT')
            zT = pp.tile([128, 2, TL], dt.float32r, name='zT')
            out_sb = pp.tile([128, TL], dt.float32, name='out_sb')

            nc.sync.dma_start(mtri[:], mask_tri[:])
            nc.sync.dma_start(mp0[:], mask_p0[:])
            nc.sync.dma_start(selc[:], selp[:])
            nc.sync.dma_start(xl_sb[:], xloc[:])
            nc.sync.dma_start(xsh_sb[:], xshift[:])
            nc.vector.memset(ones_f32[:], 1.0)
            nc.vector.memset(ones_mf[:], 1.0)
            nc.vector.tensor_copy(ones_mat[:], ones_mf[:])
            nc.vector.memset(eps_t[:], EPS)
            nc.vector.memset(eps_s1[:], EPS * float(2.0 ** (2 * (s1 + AH))))
            nc.vector.memset(eps_ah[:], EPS * float(2.0 ** (2 * AH)))
            nc.vector.memset(dscO[:], float(2.0 ** (-(so + AY))))
            nc.vector.memset(dscW2[:], float(2.0 ** (-s2)))
            nc.vector.memset(cAX[:], float(2.0 ** (AX + AH)))
            va = Vl
            nc.vector.memset(Vl[:, :, :, :, 64:128], 1.0)
            nc.vector.memset(Vpm[:, :, :, :, 64:128], 1.0)

            psA = None

            def ln_scale(src, nch, dexp=0):
                """[128, TL] f32 of rstd*2^-dexp for src [128, nch, TL].
                Stats are computed replicated on all 128 partitions (ones
                matrix as matmul lhsT) so no partition_broadcast is needed;
                matmul cost is per-column and DVE cost per-free-element, so
                this is free."""
                ps_su = psA.tile([128, TL], dt.float32, tag='ps', bufs=8,
                                 name='ps_su')
                for c in range(nch):
                    nc.tensor.matmul(ps_su[:], ones_mat[:], src[:, c, :],
                                     start=(c == 0), stop=(c == nch - 1))
                mu = smp.tile([128, TL], dt.float32, tag='lnmu', bufs=1,
                              name='mu')
                nc.scalar.activation(mu[:], ps_su[:], F.Copy,
                                     scale=1.0 / (nch * 128))
                ps_sq = psA.tile([128, TL], dt.float32, tag='ps', bufs=8,
                                 name='ps_sq')
                for c in range(nch):
                    sq_t = wkp.tile([128, TL], dt.float32r, tag='sq', bufs=2,
                                    name='sq')
                    nc.scalar.activation(sq_t[:], src[:, c, :], F.Square)
                    nc.tensor.matmul(ps_sq[:], ones_mat[:], sq_t[:],
                                     start=(c == 0), stop=(c == nch - 1))
                var = smp.tile([128, TL], dt.float32, tag='lnvar', bufs=1,
                               name='var')
                nc.scalar.activation(var[:], ps_sq[:], F.Copy,
                                     scale=1.0 / (nch * 128))
                scr = smp.tile([128, TL], dt.float32, tag='lnscr', bufs=1,
                               name='scr')
                nc.vector.tensor_mul(scr[:], mu[:], mu[:])
                nc.vector.tensor_sub(var[:], var[:], scr[:])
                # sqrt(2^2d * (var+eps)) = 2^d*sqrt(var+eps) -> rstd gets 2^-d
                ept = eps_t if dexp == 0 else (eps_ah if dexp == AH else eps_s1)
                nc.scalar.activation(var[:], var[:], F.Sqrt,
                                     bias=ept[:, 0:1],
                                     scale=float(2.0 ** (2 * dexp)))
                sb = wkp.tile([128, TL], dt.float32, tag='lnsb', bufs=2,
                              name='sb')
                nc.vector.reciprocal_approx_accurate(sb[:], var[:], scr[:])
                return sb

            def matphase8(src8, w_ap, l, npr_in, nch_out, out_cb, halves=1):
                """fp8 DoubleRow: out[co] = sum_p pair(w).T @ pair(src).
                halves=2 splits the output channels into two 4-bank PSUM
                groups so the phase starts while the previous phase's other
                banks still drain (same total weight bytes, half-width
                slabs)."""
                nh = nch_out // halves
                for hh in range(halves):
                    pss = [psA.tile([128, TL], dt.float32, tag='ps', bufs=8,
                                    name=f'pp{co}') for co in range(nh)]
                    for p in range(npr_in):
                        slab = wsp.tile([128, 2, nh * 128], dt.float8e4,
                                        tag='wslab8', bufs=8, name='slab')
                        wsl = w_ap[l, p] if l is not None else w_ap[p]
                        nc.sync.dma_start(
                            slab[:], wsl[:, :, hh * nh * 128:(hh + 1) * nh * 128])
                        for co in range(nh):
                            nc.tensor.matmul(pss[co][:],
                                             slab[:, :, co * 128:(co + 1) * 128],
                                             src8[:, 2 * p:2 * p + 2, :],
                                             start=(p == 0),
                                             stop=(p == npr_in - 1),
                                             perf_mode=PM.DoubleRow)
                    for co in range(nh):
                        out_cb(hh * nh + co, pss[co])

            def matphase(src, w_ap, l, nch_in, nch_out, out_cb, wtag, wdt):
                """bf16: out[co] = sum_ci w[ci,co].T @ src[:,ci,:]."""
                pss = [psA.tile([128, TL], dt.float32, tag='ps', bufs=8,
                                name=f'pp{co}') for co in range(nch_out)]
                for ci in range(nch_in):
                    slab = wsp.tile([128, nch_out * 128], wdt, tag=wtag,
                                    bufs=(4 if wtag == 'wslab' else 5), name='slab')
                    src_w = w_ap[l, ci * 128:(ci + 1) * 128, :] if l is not None \
                        else w_ap[ci * 128:(ci + 1) * 128, :]
                    nc.sync.dma_start(slab[:], src_w)
                    for co in range(nch_out):
                        nc.tensor.matmul(pss[co][:], slab[:, co * 128:(co + 1) * 128],
                                         src[:, ci, :], start=(ci == 0),
                                         stop=(ci == nch_in - 1))
                for co in range(nch_out):
                    out_cb(co, pss[co])

            def mk_h8():
                for co in range(NCH):
                    nc.vector.tensor_scalar_mul(h8[:, co, :], hT[:, co, :],
                                                float(2.0 ** AH))

            # ---------------- embed ----------------
            with tc.tile_pool(name='psE', bufs=1, space='PSUM') as psA:
                embs = wsp.tile([128, C], dt.bfloat16, tag='wslabb', bufs=5,
                                name='embs')
                nc.sync.dma_start(embs[:], emb_w[:])
                ebias = smp.tile([128, NCH], dt.float32, tag='bias8', bufs=2,
                                 name='ebias')
                nc.sync.dma_start(ebias[:], emb_b[:])
                for co in range(NCH):
                    ps = psA.tile([128, TL], dt.float32, tag='ps', bufs=8,
                                  name=f'pe{co}')
                    nc.tensor.matmul(ps[:], embs[:, co * 128:(co + 1) * 128],
                                     xl_sb[:], start=True, stop=True)
                    nc.scalar.activation(hT[:, co, :], ps[:], F.Identity,
                                         bias=ebias[:, co:co + 1])
                mk_h8()

            # ---------------- layers ----------------
            for li in range(L_RUN):
                with tc.tile_pool(name=f'psA{li}', bufs=1, space='PSUM') as psA:
                    sb1 = ln_scale(hT, NCH, dexp=AH)

                    kb = smp.tile([128, NCH], dt.float32, tag='bias8', bufs=2,
                                  name='kb')
                    if use_bias['qkv']:
                        nc.sync.dma_start(kb[:], bk[li])

                    def evict_k(co, ps):
                        nc.vector.tensor_mul(KTl[:, co, :], ps[:], sb1[:])
                        if use_bias['qkv']:
                            nc.vector.tensor_scalar_add(KTl[:, co, :], KTl[:, co, :],
                                                        kb[:, co:co + 1])
                    matphase8(h8, wk, li, NPR, NCH, evict_k, halves=2)

                    # K exchanged as 4 wave-aligned chunks (channel tiles
                    # 2g,2g+1) so attention wave g only waits on chunk g
                    outbKs = []
                    kfl = KTl[:].rearrange("p c t -> p (c t)")
                    for ch in range(4):
                        cs = slice(ch * (KW // 4), (ch + 1) * (KW // 4))
                        inbK = drp.tile([128, KW // 4], dt.bfloat16,
                                        tag='inbK', bufs=8, name='inbK')
                        outbK = drp.tile([256, KW // 4], dt.bfloat16,
                                         tag='outbK', bufs=8, name='outbK')
                        nc.sync.dma_start(inbK[:], kfl[:, cs])
                        nc.gpsimd.collective_compute(
                            "AllGather", ALU.bypass,
                            replica_groups=[[0, 1], [2, 3], [4, 5], [6, 7]],
                            ins=[inbK.opt()], outs=[outbK.opt()])
                        outbKs.append(outbK)

                    x8 = wkp.tile([128, NCH, TL], dt.float8e4, tag='xs', bufs=1,
                                  name='x8')
                    for c in range(NCH):
                        nc.vector.scalar_tensor_tensor(
                            x8[:, c, :], hT[:, c, :], cAX[:, 0:1], sb1[:],
                            ALU.mult, ALU.mult)

                    psv = [psA.tile([128, TL], dt.float32, tag='ps', bufs=8,
                                    name=f'pv{i}') for i in range(8)]
                    for p in range(NPR):
                        slab = wsp.tile([128, 2, C], dt.float8e4, tag='wslab8',
                                        bufs=8, name='vslab')
                        nc.sync.dma_start(slab[:], wv[li, p])
                        for tb in range(NB):
                            for dvh in range(2):
                                nc.tensor.matmul(
                                    psv[tb * 2 + dvh][:],
                                    x8[:, 2 * p:2 * p + 2, tb * 128:(tb + 1) * 128],
                                    slab[:, :, dvh * 512:(dvh + 1) * 512],
                                    start=(p == 0), stop=(p == NPR - 1),
                                    perf_mode=PM.DoubleRow)
                    for tb in range(NB):
                        for dvh in range(2):
                            src = psv[tb * 2 + dvh][:].rearrange(
                                "p (h e) -> p h e", e=64)
                            for gg in range(2):
                                dst = va[:, 2 * dvh + gg, tb, 0:4, 0:64]
                                nc.scalar.activation(dst, src[:, 4 * gg:4 * gg + 4,
                                                              :], F.Copy)

                    # V data cols exchanged as 4 head-group chunks (the
                    # ones padding is rebuilt locally, not transferred)
                    VW2 = NB * 4 * 64
                    outbVs = []
                    for ch in range(4):
                        inbV = drp.tile([128, VW2], dt.bfloat16,
                                        tag='inbV', bufs=8, name='inbV')
                        outbV = drp.tile([256, VW2], dt.bfloat16,
                                         tag='outbV', bufs=8, name='outbV')
                        nc.sync.dma_start(inbV[:], Vl[:, ch, :, :, 0:64])
                        nc.gpsimd.collective_compute(
                            "AllGather", ALU.bypass,
                            replica_groups=[[0, 1], [2, 3], [4, 5], [6, 7]],
                            ins=[inbV.opt()], outs=[outbV.opt()])
                        outbVs.append(outbV)

                    qb = smp.tile([128, NCH], dt.float32, tag='bias8', bufs=2,
                                  name='qb')
                    if use_bias['qkv']:
                        nc.sync.dma_start(qb[:], bq[li])

                    def evict_q(co, ps):
                        nc.vector.tensor_mul(QT[:, co, :], ps[:], sb1[:])
                        if use_bias['qkv']:
                            nc.vector.tensor_scalar_add(QT[:, co, :], QT[:, co, :],
                                                        qb[:, co:co + 1])
                    matphase8(h8, wq, li, NPR, NCH, evict_q, halves=2)

                    def do_select(outb, dst, w):
                        h1t = wkp.tile([128, KW // 4], dt.bfloat16, tag='h1',
                                       bufs=2, name='h1t')
                        nc.sync.dma_start(dst, outb[0:128, :])
                        nc.sync.dma_start(h1t[:, 0:w], outb[128:256, :])
                        nc.vector.tensor_scalar_mul(dst, dst, selc[:, 0:1])
                        nc.vector.scalar_tensor_tensor(
                            dst, h1t[:, 0:w], selc[:, 1:2], dst,
                            ALU.mult, ALU.add)
                    for ch in range(4):
                        W4 = KW // 4
                        do_select(outbKs[ch], peer[:, ch * W4:(ch + 1) * W4], W4)
                        do_select(outbVs[ch], Vpm[:, ch, :, :, 0:64], VW2)

                with tc.tile_pool(name=f'psB{li}', bufs=1, space='PSUM') as psB:
                    vbl = smp.tile([128, NCH], dt.float32, tag='bias8v', bufs=2,
                                   name='vbl')
                    if use_bias['v']:
                        nc.sync.dma_start(vbl[:], bv[li])
                    def att_pass(cp, psy, KTs, vaa, src_i):
                        for j in range(NB):
                            qs = 128 * j
                            qn = TL - qs
                            pssc = psB.tile([128, 2, TL], dt.float32, tag='pssc',
                                            bufs=2, name='pssc')
                            for hp in range(2):
                                nc.tensor.matmul(
                                    pssc[:, hp, 0:qn],
                                    KTs[hp * 64:(hp + 1) * 64, cp,
                                        j * 128:(j + 1) * 128],
                                    QT[hp * 64:(hp + 1) * 64, cp, qs:TL],
                                    start=True, stop=True)
                            et = wkp.tile([128, 2, qn], dt.bfloat16, tag='et',
                                          bufs=3, name='et')
                            nc.scalar.activation(et[:], pssc[:, :, 0:qn], F.Exp,
                                                 scale=exp_scale)
                            msk = mtri if src_i == 0 else mp0
                            for hp in range(2):
                                nc.vector.tensor_mul(et[:, hp, 0:128],
                                                     et[:, hp, 0:128], msk[:])
                            for hp in range(2):
                                nc.tensor.matmul(
                                    psy[:, hp, qs:TL],
                                    vaa[:, cp // 2, j, 2 * (cp % 2) + hp, :],
                                    et[:, hp, :],
                                    start=(src_i == 0 and j == 0),
                                    stop=(src_i == 1 and j == NB - 1))

                    def att_evict(cp, psy):
                        ysb = wkp.tile([64, 2, TL], dt.bfloat16, tag='ysb', bufs=1,
                                       name='ysb')
                        nc.scalar.activation(ysb[:], psy[0:64, :, :], F.Copy)
                        # psy[64:128] holds the softmax denominator replicated
                        # across 64 partitions (ones columns in V's lhsT);
                        # fold V's 2^(sv+AX) scale and y8's 2^AY prescale into
                        # the reciprocal — all plain [64,2,TL] vector ops
                        rbd = wkp.tile([64, 2, TL], dt.float32, tag='rbd',
                                       bufs=1, name='rbd')
                        nc.vector.tensor_scalar_mul(rbd[:], psy[64:128, :, :],
                                                    float(2.0 ** (sv + AX - AY)))
                        rb = wkp.tile([64, 2, TL], dt.float32, tag='rb', bufs=1,
                                      name='rb')
                        rbs = wkp.tile([64, 2, TL], dt.float32, tag='rbd2',
                                       bufs=1, name='rbs')
                        nc.vector.reciprocal_approx_accurate(rb[:], rbd[:],
                                                             rbs[:])
                        for hp in range(2):
                            nc.vector.tensor_mul(y8[hp * 64:(hp + 1) * 64, cp, :],
                                                 ysb[:, hp, :], rb[:, hp, :])
                            if use_bias['v']:
                                nc.vector.tensor_scalar_add(
                                    y8[hp * 64:(hp + 1) * 64, cp, :],
                                    y8[hp * 64:(hp + 1) * 64, cp, :],
                                    vbl[hp * 64:(hp + 1) * 64, cp:cp + 1])

                    # waves of 2 head-pairs: both local passes run before the
                    # peer passes so local compute hides AllGather latency
                    for cw in range(4):
                        psys = []
                        for ci in range(2):
                            psy = psB.tile([128, 2, TL], dt.float32, tag='psy',
                                           bufs=2, name='psy')
                            att_pass(2 * cw + ci, psy, KTl, va, 0)
                            psys.append(psy)
                        for ci in range(2):
                            att_pass(2 * cw + ci, psys[ci], KTp, Vpm, 1)
                            att_evict(2 * cw + ci, psys[ci])

                with tc.tile_pool(name=f'psC{li}', bufs=1, space='PSUM') as psA:
                    obias = smp.tile([128, NCH], dt.float32, tag='bias8o',
                                     bufs=2, name='obias')
                    if use_bias['o']:
                        nc.sync.dma_start(obias[:], bo[li])

                    def evict_proj(co, ps):
                        nc.vector.scalar_tensor_tensor(
                            hT[:, co, :], ps[:], dscO[:, 0:1], hT[:, co, :],
                            ALU.mult, ALU.add)
                        if use_bias['o']:
                            nc.vector.tensor_scalar_add(
                                hT[:, co, :], hT[:, co, :], obias[:, co:co + 1])
                    matphase8(y8, wo, li, NPR, NCH, evict_proj, halves=2)

                    b1s = smp.tile([128, 32], dt.float32, tag='b1s', bufs=2,
                                   name='b1s')
                    if use_bias['fc1']:
                        nc.sync.dma_start(b1s[:], b1[li])
                    b2s = smp.tile([128, NCH], dt.float32, tag='bias8', bufs=2,
                                   name='b2s')
                    if use_bias['fc2']:
                        nc.sync.dma_start(b2s[:], b2[li])

                    if li in mlp8:
                        mk_h8()
                        sb2 = ln_scale(hT, NCH, dexp=s1 + AH)
                        m_act = wkp.tile([128, 32, TL], dt.float8e4, tag='mact',
                                         bufs=1, name='m_act8')
                        for fog in range(4):
                            psf = [psA.tile([128, TL], dt.float32, tag='ps',
                                            bufs=8, name=f'pf{i}')
                                   for i in range(8)]
                            for p in range(NPR):
                                slab = wsp.tile([128, 2, 1024], dt.float8e4,
                                                tag='wslab8', bufs=8,
                                                name='f1slab')
                                nc.sync.dma_start(slab[:], w1[li, p, fog])
                                for fo in range(8):
                                    nc.tensor.matmul(
                                        psf[fo][:],
                                        slab[:, :, fo * 128:(fo + 1) * 128],
                                        h8[:, 2 * p:2 * p + 2, :],
                                        start=(p == 0), stop=(p == NPR - 1),
                                        perf_mode=PM.DoubleRow)
                            for fo in range(8):
                                fi = fog * 8 + fo
                                mtmp = wkp.tile([128, TL], dt.bfloat16,
                                                tag='mtmp', bufs=2, name='mtmp')
                                nc.vector.tensor_mul(mtmp[:], psf[fo][:], sb2[:])
                                if use_bias['fc1']:
                                    nc.vector.tensor_scalar_add(
                                        mtmp[:], mtmp[:], b1s[:, fi:fi + 1])
                                nc.scalar.activation(m_act[:, fi, :], mtmp[:],
                                                     F.Gelu)
                        for coh in range(2):
                            psm = [psA.tile([128, TL], dt.float32, tag='ps',
                                            bufs=8, name=f'pm{i}')
                                   for i in range(4)]
                            for p in range(16):
                                slab = wsp.tile([128, 2, C // 2], dt.float8e4,
                                                tag='wslab8', bufs=8,
                                                name='f2slab')
                                nc.sync.dma_start(
                                    slab[:],
                                    w2[li, p, :, :, coh * 512:(coh + 1) * 512])
                                for c4 in range(4):
                                    nc.tensor.matmul(
                                        psm[c4][:],
                                        slab[:, :, c4 * 128:(c4 + 1) * 128],
                                        m_act[:, 2 * p:2 * p + 2, :],
                                        start=(p == 0), stop=(p == 15),
                                        perf_mode=PM.DoubleRow)
                            for c4 in range(4):
                                co = coh * 4 + c4
                                nc.vector.scalar_tensor_tensor(
                                    hT[:, co, :], psm[c4][:], dscW2[:, 0:1],
                                    hT[:, co, :], ALU.mult, ALU.add)
                                if use_bias['fc2']:
                                    nc.vector.tensor_scalar_add(
                                        hT[:, co, :], hT[:, co, :],
                                        b2s[:, co:co + 1])
                    else:
                        # bf16 MLP layer (precision recovery)
                        hTb_l = wkp.tile([128, NCH, TL], dt.bfloat16, tag='hTb',
                                         bufs=1, name='hTb_l')
                        for co in range(NCH):
                            nc.vector.tensor_copy(hTb_l[:, co, :], hT[:, co, :])
                        sb2 = ln_scale(hT, NCH)
                        m16 = wkp.tile([128, 32, TL], dt.bfloat16, tag='mact',
                                       bufs=1, name='m16')
                        lb = l16idx[li]
                        for fog in range(4):
                            psf = [psA.tile([128, TL], dt.float32, tag='ps',
                                            bufs=8, name=f'pf{i}')
                                   for i in range(8)]
                            for ci in range(NCH):
                                slab = wsp.tile([128, C], dt.bfloat16,
                                                tag='wslabb', bufs=5,
                                                name='f1slabb')
                                nc.sync.dma_start(
                                    slab[:], w1b[lb, ci * 128:(ci + 1) * 128,
                                                 fog * 1024:(fog + 1) * 1024])
                                for fo in range(8):
                                    nc.tensor.matmul(
                                        psf[fo][:],
                                        slab[:, fo * 128:(fo + 1) * 128],
                                        hTb_l[:, ci, :], start=(ci == 0),
                                        stop=(ci == NCH - 1))
                            for fo in range(8):
                                fi = fog * 8 + fo
                                mtmp = wkp.tile([128, TL], dt.bfloat16,
                                                tag='mtmp', bufs=2, name='mtmp')
                                nc.vector.tensor_mul(mtmp[:], psf[fo][:], sb2[:])
                                if use_bias['fc1']:
                                    nc.vector.tensor_scalar_add(
                                        mtmp[:], mtmp[:], b1s[:, fi:fi + 1])
                                nc.scalar.activation(m16[:, fi, :], mtmp[:],
                                                     F.Gelu)
                        for coh in range(2):
                            psm = [psA.tile([128, TL], dt.float32, tag='ps',
                                            bufs=8, name=f'pm{i}')
                                   for i in range(4)]
                            for fi in range(32):
                                slab = wsp.tile([128, C // 2], dt.bfloat16,
                                                tag='wslabb', bufs=5,
                                                name='f2slabb')
                                nc.sync.dma_start(
                                    slab[:], w2b[lb, fi * 128:(fi + 1) * 128,
                                                 coh * 512:(coh + 1) * 512])
                                for c4 in range(4):
                                    nc.tensor.matmul(
                                        psm[c4][:],
                                        slab[:, c4 * 128:(c4 + 1) * 128],
                                        m16[:, fi, :], start=(fi == 0),
                                        stop=(fi == 31))
                            for c4 in range(4):
                                co = coh * 4 + c4
                                nc.vector.scalar_tensor_tensor(
                                    hT[:, co, :], psm[c4][:], ones_f32[:, 0:1],
                                    hT[:, co, :], ALU.mult, ALU.add)
                                if use_bias['fc2']:
                                    nc.vector.tensor_scalar_add(
                                        hT[:, co, :], hT[:, co, :],
                                        b2s[:, co:co + 1])
                    if li < L_RUN - 1:
                        mk_h8()

            # ---------------- head ----------------
            with tc.tile_pool(name='psH', bufs=1, space='PSUM') as psA:
                hTb = wkp.tile([128, NCH, TL], dt.bfloat16, tag='hTb', bufs=1,
                               name='hTb')
                for co in range(NCH):
                    nc.vector.tensor_copy(hTb[:, co, :], hT[:, co, :])
                sbf = ln_scale(hT, NCH)
                pbias = smp.tile([128, 2], dt.float32, tag='bias2', bufs=2,
                                 name='pbias')
                if use_bias['pw']:
                    nc.sync.dma_start(pbias[:], pb[:])

                def evict_pw(co, ps):
                    nc.vector.tensor_mul(featsT[:, co, :], ps[:], sbf[:])
                    if use_bias['pw']:
                        nc.vector.tensor_scalar_add(featsT[:, co, :],
                                                    featsT[:, co, :],
                                                    pbias[:, co:co + 1])
                matphase(hTb, pw, None, NCH, 2, evict_pw, 'wslabb', dt.bfloat16)

                d1b = smp.tile([128, 2], dt.float32, tag='bias2', bufs=2, name='d1b')
                if use_bias['dw1']:
                    nc.sync.dma_start(d1b[:], db1[:])

                def evict_d1(co, ps):
                    nc.scalar.activation(zT[:, co, :], ps[:], F.Tanh,
                                         bias=(d1b[:, co:co + 1]
                                               if use_bias['dw1'] else 0.0))
                matphase(featsT, dw1, None, 2, 2, evict_d1, 'wslabb', dt.bfloat16)

                sbz = ln_scale(zT, 2)
                zb = wkp.tile([128, 2, TL], dt.bfloat16, tag='zb', bufs=1, name='zb')
                for co in range(2):
                    nc.vector.tensor_copy(zb[:, co, :], zT[:, co, :])
                d2b = smp.tile([128, 1], dt.float32, tag='bias2', bufs=2, name='d2b')
                if use_bias['dw2']:
                    nc.sync.dma_start(d2b[:], db2[:])

                def evict_out(co, ps):
                    ptmp = wkp.tile([128, TL], dt.float32, tag='ptmp', bufs=1,
                                    name='ptmp')
                    nc.vector.tensor_mul(ptmp[:], ps[:], sbz[:])
                    if use_bias['dw2']:
                        nc.vector.scalar_tensor_tensor(out_sb[:], ptmp[:],
                                                       d2b[:, 0:1], xsh_sb[:],
                                                       ALU.add, ALU.subtract)
                    else:
                        nc.vector.tensor_sub(out_sb[:], ptmp[:], xsh_sb[:])
                matphase(zb, dw2, None, 2, 1, evict_out, 'wslabb', dt.bfloat16)
                nc.sync.dma_start(out_p[:], out_sb[:])

    nc.compile()
    return nc


def _fold(g, b, W, bias, scl=1.0):
    """LN(x;g,b) @ W + bias == (x @ W'')*rstd + v with the mean folded in."""
    g = np.asarray(g, np.float64)
    W = np.asarray(W, np.float64)
    u = g @ W
    Wf = (g[:, None] * W - u[None, :] / W.shape[0]) * scl
    v = (np.asarray(b, np.float64) @ W + np.asarray(bias, np.float64)) * scl
    return Wf.astype(AF32), v.astype(AF32)


def _r8(v):
    return np.ascontiguousarray(np.asarray(v, AF32).reshape(-1, 128).T)


def _sexp(absmax):
    """Largest s with absmax * 2^s <= 240."""
    return int(np.floor(np.log2(240.0 / max(absmax, 1e-30))))


def _pack8(W, s):
    """[Cin, Cout] f32 -> [Cin//256, 128, 2, Cout] fp8e4 scaled by 2^s."""
    cin, cout = W.shape
    Wr = (np.asarray(W, AF32) * np.float32(2.0 ** s)).reshape(
        cin // 256, 2, 128, cout).transpose(0, 2, 1, 3)
    return np.ascontiguousarray(Wr).astype(AE4)


def kernel(**inputs):
    x = np.asarray(inputs['tokens'], AF32).reshape(B, T, DIN)
    ln1_g, ln1_b = inputs['ln1_g'], inputs['ln1_b']
    ln2_g, ln2_b = inputs['ln2_g'], inputs['ln2_b']
    scale = 1.0 / np.sqrt(C // H)

    wqf = np.empty((L, C, C), AF32)
    wkf = np.empty((L, C, C), AF32)
    wvf = np.empty((L, C, C), AF32)
    w1f = np.empty((L, C, DFF), AF32)
    bqh = np.empty((L, 128, NCH), AF32)
    bkh = np.empty((L, 128, NCH), AF32)
    bvh = np.empty((L, 128, NCH), AF32)
    boh = np.empty((L, 128, NCH), AF32)
    b1h = np.empty((L, 128, 32), AF32)
    b2h = np.empty((L, 128, NCH), AF32)
    for l in range(L):
        Wf, v = _fold(ln1_g[l], ln1_b[l], inputs['Wq'][l], inputs['bq'][l], scale)
        wqf[l], bqh[l] = Wf, _r8(v)
        Wf, v = _fold(ln1_g[l], ln1_b[l], inputs['Wk'][l], inputs['bk'][l])
        wkf[l], bkh[l] = Wf, _r8(v)
        Wf, v = _fold(ln1_g[l], ln1_b[l], inputs['Wv'][l], inputs['bv'][l])
        wvf[l], bvh[l] = Wf, _r8(v)
        boh[l] = _r8(inputs['bo'][l])
        Wf, v = _fold(ln2_g[l], ln2_b[l], inputs['W1'][l], inputs['b1'][l])
        w1f[l], b1h[l] = Wf, _r8(v)
        b2h[l] = _r8(inputs['b2'][l])
    wof = np.asarray(inputs['Wo'], AF32)
    w2f = np.asarray(inputs['W2'], AF32)

    sq = _sexp(np.abs(wqf).max())
    sk = _sexp(np.abs(wkf).max())
    sv = _sexp(np.abs(wvf).max())
    so = _sexp(np.abs(wof).max())
    s1 = _sexp(np.abs(w1f).max())
    s2 = _sexp(np.abs(w2f).max())

    wq8 = np.empty((L, NPR, 128, 2, C), AE4)
    wk8 = np.empty((L, NPR, 128, 2, C), AE4)
    wv8 = np.empty((L, NPR, 128, 2, C), AE4)
    wo8 = np.empty((L, NPR, 128, 2, C), AE4)
    w18 = np.empty((L, NPR, 4, 128, 2, 1024), AE4)
    w28 = np.empty((L, DFF // 256, 128, 2, C), AE4)
    for l in range(L):
        wq8[l] = _pack8(wqf[l], sq)
        wk8[l] = _pack8(wkf[l], sk)
        wv8[l] = _pack8(wvf[l], sv)
        wo8[l] = _pack8(wof[l], so)
        w18[l] = _pack8(w1f[l], s1).reshape(NPR, 128, 2, 4, 1024) \
            .transpose(0, 3, 1, 2, 4)
        w28[l] = _pack8(w2f[l], s2)

    # K/Q biases ride on the 2^s-scaled K/Q tensors
    bqh *= np.float32(2.0 ** sq)
    bkh *= np.float32(2.0 ** sk)

    pwf, pv = _fold(inputs['lnf_g'], inputs['lnf_b'], inputs['proj_w'],
                    inputs['proj_b'])
    dw2f, d2v = _fold(inputs['dec_ln_g'], inputs['dec_ln_b'], inputs['dec_w2'],
                      inputs['dec_b2'])
    d1v = np.asarray(inputs['dec_b1'], AF32)

    use_bias = {
        'qkv': bool(np.abs(bqh).max() > 0 or np.abs(bkh).max() > 0),
        'v': bool(np.abs(bvh).max() > 0),
        'o': bool(np.abs(boh).max() > 0),
        'fc1': bool(np.abs(b1h).max() > 0),
        'fc2': bool(np.abs(b2h).max() > 0),
        'pw': bool(np.abs(pv).max() > 0),
        'dw1': bool(np.abs(d1v).max() > 0),
        'dw2': bool(np.abs(d2v).max() > 0),
    }
    key = (tuple(sorted(use_bias.items())), sq, sk, sv, so, s1, s2,
           tuple(sorted(MLP8)))
    if key not in _CACHE:
        _CACHE[key] = _build(use_bias, sq, sk, sv, so, s1, s2, MLP8)
    nc = _CACHE[key]
    l16 = sorted(set(range(L)) - MLP8)

    tri = np.tril(np.ones((128, 128), AF32)).T.astype(ABF16)
    shared = dict(
        emb_w=np.asarray(inputs['tok_emb_w'], AF32).astype(ABF16),
        emb_b=_r8(inputs['tok_emb_b']),
        wq=wq8, wk=wk8, wv=wv8, wo=wo8, w1=w18, w2=w28,
        bq=bqh, bk=bkh, bv=bvh, bo=boh, b1=b1h, b2=b2h,
        pw=pwf.astype(ABF16), pb=_r8(pv),
        dw1=np.asarray(inputs['dec_w1'], AF32).astype(ABF16), db1=_r8(d1v),
        dw2=dw2f.astype(ABF16), db2=np.ascontiguousarray(d2v.reshape(1, 128).T),
        mask_tri=tri,
    )
    if l16:
        shared['w1b'] = w1f[l16].astype(ABF16)
        shared['w2b'] = w2f[l16].astype(ABF16)

    in_maps = []
    for c in range(8):
        b_, s = c // 2, c % 2
        tloc = np.concatenate([np.arange(128 * (2 * j + s), 128 * (2 * j + s) + 128)
                               for j in range(NB)])
        tnext = np.minimum(tloc + 1, T - 1)
        im = dict(shared)
        im['xloc'] = np.ascontiguousarray(x[b_][tloc].T).astype(ABF16)
        im['xshift'] = np.ascontiguousarray(x[b_][tnext].T)
        im['mask_p0'] = np.full((128, 128), float(s), AF32).astype(ABF16)
        im['selp'] = np.tile(np.array([[float(s), 1.0 - s]], AF32), (128, 1))
        in_maps.append(im)

    res = run_bass_kernel_spmd(nc, in_maps, list(range(8)))
    out = np.empty((B, T - 1, DIN), AF32)
    for c in range(8):
        b_, s = c // 2, c % 2
        o = res.results[c]['out'].T  # [TL, 128], DMA'd untransposed
        for j in range(NB):
            g = 2 * j + s
            t0, t1 = 128 * g, min(128 * g + 128, T - 1)
            out[b_, t0:t1] = o[128 * j:128 * j + (t1 - t0)]
    return out
